# revision 25
# baseline (speedup 1.0000x reference)
"""Trainium2 Bass kernel for CondLaneRNNHead-style dynamic mask head.

Computation (see reference): per-instance 3-layer 1x1-conv MLP over
per-image feature maps augmented with 2 coordinate channels.

  out[m] = w2[m] @ relu(w1[m] @ relu(w0[m] @ [coords; x[img(m)]] + b0[m]) + b1[m]) + b2[m]

Shapes: x [4, 64, 80, 200] f32, mask_head_params [32, 8513] f32, num_ins=8.
Output [1, 32, 80, 200] f32.

Sharding: spatial, along H. Core k processes rows [10k, 10k+10) of all 4
images for all 32 instances.

v4 -- v2's software-pipelined structure + PE col-tiling for layer 2.
trn2's PE runs matmuls at DISJOINT 32x32 cell rectangles concurrently
(tile_position col groups; measured: 4 K=128/M=2 col tiles sustain 216ns
per 512-col group = full 4x overlap). stage2's four pair-matmuls per
quad-chunk are emitted as M=2 tiles at col positions {0,32,64,96} into
one PSUM tile: ~0.55us/quad instead of ~2.1us, saving ~11us of PE time.

Layer 1 deliberately stays as v2's single block-diagonal K=128 matmul
per pair, NOT quadrant-packed: a packed variant (v3) dropped PE busy to
~55% of the span and the PE_HAM activity monitor demoted the PE clock
to 1.2 GHz for the whole main loop (58us at K=4/8, +14us total). The PE
must stay ~95% duty to hold 2.4 GHz, so PE work is kept at ~= the
PSUM-drain time. The drain engines (ACT (172+FD)/1.2ns, DVE
(120+FD)/0.96ns per op, fp32 PSUM src is 1x on both; only these two
engines can read PSUM) are the floor at ~43us busy each; relus/moves
are assigned by greedy projected-busy balance (ACT is ~15% faster so
it takes ~35 of 64 relus) instead of v2's strict alternation.

Kept from v2: fp8 feats (no pad rows at K=66), one-tensor biases,
image-major DMA spreading across the gpsimd/sync/scalar queues, PE
clock warmup matmuls during the input-DMA wait, LDWEIGHTS dedupe
(upgraded: footprint-aware, so repeat loads at a tile_position whose
cells are untouched since the last identical load are dropped even
with other-position loads interposed).
"""

import numpy as np
from contextlib import ExitStack

N_IMG, C, H, W = 4, 64, 80, 200
NUM_INS = 8
M = N_IMG * NUM_INS          # 32 instances
N_CORES = 8
HPC = H // N_CORES           # 10 rows of H per core
SPI = HPC * W                # 2000 spatial positions per image slice
PAIRS = M // 2               # 16
CH = C + 2                   # 66 input channels incl. coords
FD = 1000                    # activation chunk
# matmul free-dim splits inside each 1000 chunk: PSUM banks hold 512 f32, and
# a matmul output must not cross a bank boundary -> split 512 + 488.
SPLITS = ((0, 512), (512, 488))
FDP = 1024                   # padded stride in the quad PSUM tile

_W0N, _W1N, _W2N = CH * C, C * C, C
_B2_SHIFT = -2.19

# stage1 groups (keyed by (image, pair-group)) that run quadrant-PACKED:
# two pairs in one concurrent 4-tile pass (2x PE throughput). All groups
# are packed; the PE duty lost to packing is backfilled with parasite
# filler matmuls (see stage2_batch) so the PE_HAM activity monitor never
# demotes the clock to 1.2 GHz (v3/v5/v7 regressions: once the PE goes
# cold mid-loop it never re-warms -- the pipeline's micro-gaps keep the
# activity window from ever reading "busy" again).
# The packed group's odd pair lands half-swapped in PSUM; the host-side
# b1/l2 packing compensates (must agree with this set).
_PACKED = {(i, g) for i in range(4) for g in range(2)}
# parasite filler matmuls per quad-chunk: 256-col wsrc matmuls written
# into the psq tile's dead window (after the out-move reads, before the
# next quad's layer-2 matmuls). ~110ns of guaranteed PE busy each.
_FILL = 16


def _act_cost(fd):
    return (172 + fd) / 1.2


def _dve_cost(fd):
    return (120 + fd) / 0.96


_COMPILED = {}


def _build_program():
    import concourse.bacc as bacc
    import concourse.tile as tile
    from concourse import mybir

    dt = mybir.dt
    AF = mybir.ActivationFunctionType
    OP = mybir.AluOpType

    nc = bacc.Bacc("TRN2", target_bir_lowering=False, debug=False)

    # feats in fp8 (normal-mode matmul, bf16 weights): halves the feats DMA
    # and, at K=66, partitions 66:128 are never read by the PE, so no
    # zero-padding or pad memsets are needed at all.
    xs_d = nc.dram_tensor("xs", [N_IMG, CH, SPI], dt.float8e4, kind="ExternalInput").ap()
    l0_d = nc.dram_tensor("l0t", [N_IMG, CH, 512], dt.bfloat16, kind="ExternalInput").ap()
    # layer1 weights, dense per-instance 64x64 blocks (quadrant layout):
    # per image [128, 256]; pair-group g (pairs 4i+2g, 4i+2g+1) at cols 128g:
    #   rows 0:64   = [w1T(pair0, instA) | w1T(pair1, instA)]
    #   rows 64:128 = [w1T(pair1, instB) | w1T(pair0, instB)]
    # Serves both modes: an unpacked pair runs as its two diagonal tiles
    # (same wall time as v2's block-diagonal K=128 matmul), a packed group
    # runs all four tiles of two pairs concurrently.
    l1_d = nc.dram_tensor("l1t", [N_IMG, 128, 256], dt.bfloat16, kind="ExternalInput").ap()
    # layer2 weights: [128, 2] per pair (col r -> psq row 32j+r).
    l2_d = nc.dram_tensor("l2t", [128, PAIRS * 2], dt.bfloat16, kind="ExternalInput").ap()
    # all biases in one tensor/one DMA.
    ball_d = nc.dram_tensor("ball", [128, 2 * PAIRS + 4], dt.float32, kind="ExternalInput").ap()
    # out[q, j, r, :] = instance 8q + 2j + r, i.e. plain instance-major order
    out_d = nc.dram_tensor("out", [4, 4, 2, SPI], dt.float32, kind="ExternalOutput").ap()

    f32 = dt.float32
    bf16 = dt.bfloat16
    f8 = dt.float8e4

    with tile.TileContext(nc) as tc, ExitStack() as ctx:
        cpool = ctx.enter_context(tc.tile_pool(name="const", bufs=1))
        h1pool = ctx.enter_context(tc.tile_pool(name="h1p", bufs=4))
        h2pool = ctx.enter_context(tc.tile_pool(name="h2p", bufs=6))
        oqpool = ctx.enter_context(tc.tile_pool(name="oqp", bufs=4))
        pspool = ctx.enter_context(tc.tile_pool(name="ps", bufs=3, space="PSUM"))
        psqpool = ctx.enter_context(tc.tile_pool(name="psq", bufs=1, space="PSUM"))

        # ---- resident tiles ----
        ball = cpool.tile([128, 2 * PAIRS + 4], f32, tag="ball", name="ball")

        def b0ap(p):
            return ball[:, p : p + 1]

        def b1ap(p):
            return ball[:, PAIRS + p : PAIRS + p + 1]

        def b2ap(i):
            return ball[:, 2 * PAIRS + i : 2 * PAIRS + i + 1]

        l2s = cpool.tile([128, PAIRS * 2], bf16, tag="l2s", name="l2s")
        fe = [cpool.tile([CH, SPI], f8, tag=f"fe{n}", name=f"fe{n}")
              for n in range(N_IMG)]
        l0s = [cpool.tile([CH, 512], bf16, tag=f"l0g{n}", name=f"l0g{n}")
               for n in range(N_IMG)]
        l1s = [cpool.tile([128, 256], bf16, tag=f"l1g{n}", name=f"l1g{n}")
               for n in range(N_IMG)]

        # ---- input loads, image-major, spread across the three DMA-capable
        # engine queues. scalar (ACT) gets ONLY image-0's two jobs: each
        # dma_start costs ~0.7us of engine time and ACT must be free for
        # relus as soon as the pipeline starts.
        # The first sync job is image 0's first 1024-column chunk (all 66
        # rows) so units 0-3 can start ~1.5us before the full image lands;
        # the remaining image-0 jobs cover only cols 1024: so there is no
        # overlap-WAR on the fast path.
        wsrc = cpool.tile([128, 640], bf16, tag="wsrc", name="wsrc")
        nc.vector.memset(wsrc[:], 0.0)
        nc.scalar.dma_start(fe[0][33:CH, 1024:SPI], xs_d[0, 33:CH, 1024:SPI])
        nc.scalar.dma_start(l1s[0][0:64, :], l1_d[0, 0:64])
        nc.sync.dma_start(fe[0][:, 0:1024], xs_d[0, :, 0:1024])
        for n in range(N_IMG):
            nc.sync.dma_start(l0s[n][:], l0_d[n])
            if n == 0:
                nc.sync.dma_start(fe[0][0:33, 1024:SPI], xs_d[0, 0:33, 1024:SPI])
            else:
                nc.sync.dma_start(fe[n][0:33, :], xs_d[n, 0:33, :])
                nc.sync.dma_start(fe[n][33:CH, :], xs_d[n, 33:CH, :])
        nc.gpsimd.dma_start(ball[:], ball_d[:])
        nc.gpsimd.dma_start(l1s[0][64:128, :], l1_d[0, 64:128])
        nc.gpsimd.dma_start(l1s[1][0:64, :], l1_d[1, 0:64])
        nc.gpsimd.dma_start(l2s[:], l2_d[:])
        nc.gpsimd.dma_start(l1s[1][64:128, :], l1_d[1, 64:128])
        nc.gpsimd.dma_start(l1s[2][:], l1_d[2])
        nc.gpsimd.dma_start(l1s[3][:], l1_d[3])

        # ---- PE clock warmup ----
        # The PE_HAM clock gate passes 4/8 pulses (1.2 GHz) until the PE has
        # been busy for a full free-running ~3.4us activity window. These
        # dependency-free matmuls run during the input-DMA wait and promote
        # the clock to 2.4 GHz before the first real matmul. 20 x 256 cols
        # = ~4.3us cold: one full activity window plus a bridge over the
        # data-arrival jitter (a ~1us PE idle right after warmup was enough
        # to re-throttle the clock for the whole run on some devices).
        wps = pspool.tile([128, FD], f32, tag="ps", name="wps")
        for _ in range(20):
            nc.tensor.matmul(
                wps[:, 0:256], wsrc[:, 0:128], wsrc[:, 128:384],
                start=True, stop=True,
            )

        # ---- relu engine assignment: greedy balance of projected busy ----
        eng_busy = {"act": 0.0, "dve": 0.0}

        def pick_engine(fd):
            ta = eng_busy["act"] + _act_cost(fd)
            td = eng_busy["dve"] + _dve_cost(fd)
            if ta <= td:
                eng_busy["act"] = ta
                return nc.scalar
            eng_busy["dve"] = td
            return nc.vector

        # ---- software-pipelined main loop ----
        # units: (image, chunk, pair-in-image), chunk-major inside an image
        # so a quad-chunk's four layer-2 matmuls are consecutive.
        units = [(i, h, j) for i in range(N_IMG) for h in range(2) for j in range(4)]
        U = len(units)
        h1t = [None] * U
        h2t = [None] * U

        def emit_relu(dst, src, bias):
            eng = pick_engine(FD)
            if eng is nc.scalar:
                nc.scalar.activation(dst, src, AF.Relu, bias=bias)
            else:
                eng.tensor_scalar(dst, src, bias, 0.0, OP.add, OP.max)

        def stage0(s):
            i, h, j = units[s]
            p = 4 * i + j
            base = h * FD
            w0 = l0s[i][:, j * 128 : (j + 1) * 128]
            ps0 = pspool.tile([128, FD], f32, tag="ps", name=f"ps0_{s}")
            for off, sz in SPLITS:
                nc.tensor.matmul(
                    ps0[:, off : off + sz],
                    w0,
                    fe[i][:, base + off : base + off + sz],
                    start=True,
                    stop=True,
                )
            h1 = h1pool.tile([128, FD], bf16, tag="h1", name=f"h1_{s}")
            emit_relu(h1[:], ps0[:], b0ap(p))
            h1t[s] = h1

        def stage1_single(s):
            # one pair as its two diagonal K=64/M=64 tiles (concurrent, same
            # wall time as a block-diagonal K=128 matmul). In the quadrant
            # layout the odd pair's instA/instB columns are crosswise.
            i, h, j = units[s]
            p = 4 * i + j
            gg = j // 2
            wbase = 128 * gg
            odd = j % 2
            wA = l1s[i][0:64, wbase + 64 * odd : wbase + 64 * odd + 64]
            wB = l1s[i][64:128, wbase + 64 * (1 - odd) : wbase + 64 * (1 - odd) + 64]
            ps1 = pspool.tile([128, FD], f32, tag="ps", name=f"ps1_{s}")
            r = h1t[s]
            for off, sz in SPLITS:
                sl = slice(off, off + sz)
                nc.tensor.matmul(ps1[0:64, sl], wA, r[0:64, sl],
                                 start=True, stop=True, tile_position=(0, 0))
                nc.tensor.matmul(ps1[64:128, sl], wB, r[64:128, sl],
                                 start=True, stop=True, tile_position=(64, 64))
            h2 = h2pool.tile([128, FD], bf16, tag="h2", name=f"h2_{s}")
            emit_relu(h2[:], ps1[:], b1ap(p))
            h2t[s] = h2

        def stage1_packed(s):
            # quadrant-packed layer1 for units s (pair p0) and s+1 (pair p1):
            # 4 concurrent K=64/M=64 tiles, 2 split matmuls each. Pair p1's
            # two instances land half-swapped in psB; the host-side b1/l2
            # packing compensates.
            i, h, j = units[s]
            gg = j // 2
            p0, p1 = 4 * i + j, 4 * i + j + 1
            wbase = 128 * gg
            wA0 = l1s[i][0:64, wbase : wbase + 64]          # pair p0 instA
            wB0 = l1s[i][64:128, wbase + 64 : wbase + 128]  # pair p0 instB
            wA1 = l1s[i][0:64, wbase + 64 : wbase + 128]    # pair p1 instA
            wB1 = l1s[i][64:128, wbase : wbase + 64]        # pair p1 instB
            psA = pspool.tile([128, FD], f32, tag="ps", name=f"ps1a_{s}")
            psB = pspool.tile([128, FD], f32, tag="ps", name=f"ps1b_{s}")
            r0, r1 = h1t[s], h1t[s + 1]
            for off, sz in SPLITS:
                sl = slice(off, off + sz)
                nc.tensor.matmul(psA[0:64, sl], wA0, r0[0:64, sl],
                                 start=True, stop=True, tile_position=(0, 0))
                nc.tensor.matmul(psA[64:128, sl], wB0, r0[64:128, sl],
                                 start=True, stop=True, tile_position=(64, 64))
                nc.tensor.matmul(psB[64:128, sl], wA1, r1[0:64, sl],
                                 start=True, stop=True, tile_position=(0, 64))
                nc.tensor.matmul(psB[0:64, sl], wB1, r1[64:128, sl],
                                 start=True, stop=True, tile_position=(64, 0))
            hA = h2pool.tile([128, FD], bf16, tag="h2", name=f"h2_{s}")
            hB = h2pool.tile([128, FD], bf16, tag="h2", name=f"h2_{s + 1}")
            emit_relu(hA[:], psA[:], b1ap(p0))
            emit_relu(hB[:], psB[:], b1ap(p1))
            h2t[s], h2t[s + 1] = hA, hB

        def stage2_batch(i, h):
            # col-packed layer2: 4 pairs' K=128/M=2 matmuls at col positions
            # {0,32,64,96} run concurrently in the PE array; emit the four
            # 512-split matmuls adjacent, then the four 488 splits.
            base = h * FD
            psq = psqpool.tile([128, FDP], f32, tag="psq", name=f"psq_{i}_{h}")
            for off, sz in SPLITS:
                for j in range(4):
                    s = 8 * i + 4 * h + j
                    p = 4 * i + j
                    nc.tensor.matmul(
                        psq[32 * j : 32 * j + 2, off : off + sz],
                        l2s[:, 2 * p : 2 * p + 2],
                        h2t[s][:, off : off + sz],
                        start=True,
                        stop=True,
                        tile_position=(0, 32 * j),
                    )
            oq = oqpool.tile([128, FD], f32, tag="oq", name=f"oq_{i}_{h}")
            # one full-FD move (one per-op overhead instead of two; the
            # drain engines are the pacing constraint once stage1 is packed)
            e1 = pick_engine(FD)
            if e1 is nc.scalar:
                nc.scalar.activation(oq[:], psq[:, 0:FD], AF.Identity,
                                     bias=b2ap(i))
            else:
                nc.vector.tensor_scalar(oq[:], psq[:, 0:FD], b2ap(i),
                                        None, OP.add)
            dq = (nc.gpsimd, nc.sync) if (2 * i + h) % 2 == 0 else (nc.sync, nc.gpsimd)
            for j in range(4):
                dq[j % 2].dma_start(
                    out_d[i, j, :, base : base + FD], oq[32 * j : 32 * j + 2, :]
                )
            # parasite PE filler into psq's dead window (keeps the HAM
            # activity monitor reading "busy" while the drains pace the
            # pipeline). Skipped after the final quad.
            if (i, h) != (N_IMG - 1, 1):
                for _ in range(_FILL):
                    nc.tensor.matmul(
                        psq[:, 0:256], wsrc[:, 0:128], wsrc[:, 128:384],
                        start=True, stop=True,
                    )

        # batch emission is delayed one extra slot so the last pair's relu1
        # (which starts only mid-slot) has a full slot of PE work ahead of
        # its layer-2 matmuls.
        pending = None
        for s in range(U + 3):
            if s < U:
                stage0(s)
            u = s - 2
            if 0 <= u < U:
                i, h, j = units[u]
                gg = j // 2
                if j % 2 == 0:
                    if (i, gg) in _PACKED:
                        stage1_packed(u)
                        if j == 2:
                            pending = (i, h, s)
                    else:
                        stage1_single(u)
                else:
                    if (i, gg) not in _PACKED:
                        stage1_single(u)
                        if j == 3:
                            pending = (i, h, s)
            if pending is not None and s > pending[2]:
                stage2_batch(pending[0], pending[1])
                pending = None

    nc.compile()
    _dedupe_ldweights(nc, mybir)
    return nc


def _dedupe_ldweights(nc, mybir):
    """Drop redundant PE LDWEIGHTS after compile.

    Weights are resident per array-cell rectangle, so an LDWEIGHTS whose
    (AP, tile_position, tile_size, mode) signature matches the last retained
    LDWEIGHTS at the same position -- with no intervening load overlapping
    those cells -- is a no-op that only costs PE issue time. Skip any
    instruction carrying semaphore waits/updates so synchronization is
    untouched.
    """
    dropped = 0
    for fn in nc.m.functions:
        for blk in fn.blocks:
            new = []
            cache = {}  # pos -> (sig, rect); rect = (r0, r1, c0, c1)
            for i in blk.instructions:
                if (
                    isinstance(i, mybir.InstLdweights)
                    and i.engine == mybir.EngineType.PE
                ):
                    pos = tuple(i.tile_position or (0, 0))
                    size = tuple(i.tile_size or (128, 128))
                    rect = (pos[0], pos[0] + size[0], pos[1], pos[1] + size[1])
                    sig = (str(i.ins[0]), pos, size, i.perf_mode, i.is_transpose)
                    si = i.sync_info
                    clean = si is None or (not si.on_wait and not si.on_update)
                    ent = cache.get(pos)
                    if clean and ent is not None and ent[0] == sig:
                        dropped += 1
                        continue
                    # retained: evict any cached tile whose cells this load
                    # overwrites (overlapping row AND col ranges)
                    for k in list(cache):
                        _, (r0, r1, c0, c1) = cache[k]
                        if rect[0] < r1 and r0 < rect[1] and rect[2] < c1 and c0 < rect[3]:
                            del cache[k]
                    cache[pos] = (sig, rect)
                new.append(i)
            if dropped:
                blk.instructions.clear()
                blk.instructions.extend(new)
    return dropped


def _pack_params(mask_head_params):
    """Split generated params and build the pair-packed device layouts."""
    p = np.ascontiguousarray(mask_head_params, dtype=np.float32)
    o0, o1, o2 = _W0N, _W0N + _W1N, _W0N + _W1N + _W2N
    w0 = p[:, :o0].reshape(M, C, CH)
    w1 = p[:, o0:o1].reshape(M, C, C)
    w2 = p[:, o1:o2].reshape(M, C)
    b0 = p[:, o2 : o2 + C]
    b1 = p[:, o2 + C : o2 + 2 * C]
    b2 = p[:, o2 + 2 * C :] + np.float32(_B2_SHIFT)

    import ml_dtypes as _mld

    # layer0 lhsT: rows 0-63 = x-channel weights, 64-65 = coord weights;
    # K=66, no pad rows. Device layout [N_IMG, 66, 512], contiguous 1KB rows.
    w0T = np.transpose(w0, (2, 0, 1))              # [66(cin), 32, 64]
    w0T = np.concatenate([w0T[2:], w0T[:2]], 0)    # x channels first, coords last
    l0rows = w0T.reshape(CH, M * C)                # [66, 2048]
    l0t = np.ascontiguousarray(
        l0rows.reshape(CH, N_IMG, 512).transpose(1, 0, 2).astype(_mld.bfloat16)
    )

    # layer1: dense 64x64 per-instance blocks, quadrant layout per group of
    # 2 pairs. w1T(m) = w1[m].T ([cin, cout]).
    w1T = np.transpose(w1, (0, 2, 1)).astype(np.float32)  # [M, 64, 64]
    l1t = np.zeros((N_IMG, 128, 256), dtype=np.float32)
    for i in range(N_IMG):
        for g in range(2):
            m0 = 2 * (4 * i + 2 * g)      # even instance of pair 4i+2g
            m1 = m0 + 2                   # even instance of pair 4i+2g+1
            cb = 128 * g
            l1t[i, 0:64, cb : cb + 64] = w1T[m0]              # pair0 instA
            l1t[i, 64:128, cb + 64 : cb + 128] = w1T[m0 + 1]  # pair0 instB
            l1t[i, 0:64, cb + 64 : cb + 128] = w1T[m1]        # pair1 instA
            l1t[i, 64:128, cb : cb + 64] = w1T[m1 + 1]        # pair1 instB
    l1t = np.ascontiguousarray(l1t.astype(_mld.bfloat16))

    def _swapped(p_):
        # odd pair of a PACKED group lands half-swapped in PSUM
        return p_ % 2 == 1 and (p_ // 4, (p_ % 4) // 2) in _PACKED

    # layer2: [128, 2] per pair; col r -> output instance 2p+r. Swapped
    # pairs get their w2 halves crosswise to match the h2 layout.
    l2t = np.zeros((128, PAIRS * 2), dtype=np.float32)
    for p_ in range(PAIRS):
        if _swapped(p_):
            l2t[64:128, 2 * p_] = w2[2 * p_]
            l2t[0:64, 2 * p_ + 1] = w2[2 * p_ + 1]
        else:
            l2t[0:64, 2 * p_] = w2[2 * p_]
            l2t[64:128, 2 * p_ + 1] = w2[2 * p_ + 1]
    l2t = np.ascontiguousarray(l2t.astype(_mld.bfloat16))

    b0t = np.concatenate([b0[0::2], b0[1::2]], 1).T  # [128,16]
    b1t = np.concatenate([b1[0::2], b1[1::2]], 1).T.copy()
    for p_ in range(PAIRS):
        if _swapped(p_):
            b1t[:, p_] = np.concatenate([b1t[64:128, p_], b1t[0:64, p_]])
    # b2 packed to match the quad PSUM layout: rows 32j+r of col q hold
    # instance 8q + 2j + r.
    b2q = np.zeros((128, 4), dtype=np.float32)
    for qq in range(4):
        for j in range(4):
            b2q[32 * j, qq] = b2[8 * qq + 2 * j, 0]
            b2q[32 * j + 1, qq] = b2[8 * qq + 2 * j + 1, 0]
    ballt = np.ascontiguousarray(
        np.concatenate([b0t, b1t, b2q], axis=1), dtype=np.float32
    )  # [128, 36]
    return l0t, l1t, l2t, ballt


def _run(x, mask_head_params, trace=False, trace_kwargs=None):
    from concourse.bass_utils import run_bass_kernel_spmd

    if "nc" not in _COMPILED:
        _COMPILED["nc"] = _build_program()
    nc = _COMPILED["nc"]

    x = np.ascontiguousarray(x, dtype=np.float32)
    l0t, l1t, l2t, ballt = _pack_params(mask_head_params)

    xx = np.tile(np.arange(W, dtype=np.float32) / W, HPC)  # [2000]
    in_maps = []
    for k in range(N_CORES):
        h0 = k * HPC
        yy = np.repeat((h0 + np.arange(HPC, dtype=np.float32)) / W, W)
        coords = np.stack([xx, yy], 0)  # [2, 2000]
        import ml_dtypes as _mld

        xsl = x[:, :, h0 : h0 + HPC, :].reshape(N_IMG, C, SPI)
        xs = np.ascontiguousarray(
            np.concatenate(
                [xsl, np.broadcast_to(coords, (N_IMG, 2, SPI))], axis=1
            ).astype(_mld.float8_e4m3fn)
        )
        in_maps.append(
            {
                "xs": xs,
                "l0t": l0t,
                "l1t": l1t,
                "l2t": l2t,
                "ball": ballt,
            }
        )

    res = run_bass_kernel_spmd(
        nc,
        in_maps,
        list(range(N_CORES)),
        trace=trace,
        **(trace_kwargs or {}),
    )

    out = np.empty((1, M, H, W), dtype=np.float32)
    for k in range(N_CORES):
        oc = res.results[k]["out"].reshape(M, HPC, W)
        out[0, :, k * HPC : (k + 1) * HPC, :] = oc
    return out, res


def kernel(x, mask_head_params, num_ins):
    n_ins = int(np.asarray(num_ins))
    assert n_ins == NUM_INS, f"kernel hardcoded for num_ins={NUM_INS}, got {n_ins}"
    out, _ = _run(x, mask_head_params)
    return out
